# revision 16
# baseline (speedup 1.0000x reference)
"""AR-LSTM sampling kernel for 8 TRN2 NeuronCores.

nn_ARLSTMModel: 1024-step autoregressive LSTM rollout (H=512, D=64, bs=256),
data-parallel over batch (32 rows/core), weights replicated in SBUF.

Math (per step, per core, feature-major):
    gates = W~ @ h + W_ih @ z + b~          (W~ = W_hh + W_ih@W_pmu folds the
                                             mu-part of the y feedback;
                                             z = eps*std is the only feedback)
    i,f,o ~ sigmoid, g ~ tanh; sigmoid computed as 0.5*tanh(x/2)+0.5 so the
    whole step uses one ACT table set (tanh+exp).
    State is stored doubled (H=2h, C=2c) so the cell update becomes fused
    scalar_tensor_tensor ops:
        A  = (tanh_i + 1) * tanh_g
        B  = (tanh_f + 1) * C
        C' = 0.5*B + A
        H' = (tanh_o + 1) * tanh(0.5*C')
    with the 0.5 factors folded into the weights host-side.
    proj: [mu|lv] = 0.5*W_proj @ H + b_proj (bias via const `1` row in the
    moving state vector s4 = [z(64); 1; 0...]).
    y = z + mu,  z' = eps_t * exp(0.5*lv)
"""

import numpy as np

H, D, BS, NT = 512, 64, 256, 1024
NCORES = 8
B = BS // NCORES  # 32
P = 128
U = 16  # steps per For_i iteration


# ---------------------------------------------------------------- weights fold
def fold_weights(W_ih, W_hh, b_ih, b_hh, W_proj, b_proj):
    """Host-side fold + layout. Returns dict of fp16 arrays for DRAM params."""
    W_ih = np.asarray(W_ih, np.float32)
    W_hh = np.asarray(W_hh, np.float32)
    W_proj = np.asarray(W_proj, np.float32)
    b_proj = np.asarray(b_proj, np.float32)
    W_pmu = W_proj[:D]          # [64, 512]
    b_mu = b_proj[:D]
    Wt = W_hh + W_ih @ W_pmu    # [2048, 512]
    bt = np.asarray(b_ih, np.float32) + np.asarray(b_hh, np.float32) + W_ih @ b_mu

    # m-tile order: m = 4q + j, j in {0:i, 1:f, 2:o, 3:g}; rows Goff_j + 128q
    goff = {0: 0, 1: H, 2: 3 * H, 3: 2 * H}  # i, f, o, g
    wh = np.zeros((P, 4, 16, P), np.float16)   # [K=128, kc, m, M=128]
    w4 = np.zeros((P, 16, P), np.float16)      # [K=128 (z,1,pad), m, M]
    for q in range(4):
        for j in range(4):
            m = 4 * q + j
            rows = slice(goff[j] + P * q, goff[j] + P * q + P)
            s = 0.5 if j < 3 else 1.0          # sigmoid-as-tanh prescale
            for kc in range(4):
                # extra 0.5: state stored as H=2h
                wh[:, kc, m, :] = (s * 0.5 * Wt[rows, P * kc:P * (kc + 1)]).T
            w4[:D, m, :] = (s * W_ih[rows, :]).T   # z enters unscaled
            w4[D, m, :] = s * bt[rows]             # bias row (hits the `1`)
    wp = np.zeros((P, 4, P), np.float16)       # proj moving [K=H chunk, kc, 2D]
    for kc in range(4):
        wp[:, kc, :] = (0.5 * W_proj[:, P * kc:P * (kc + 1)]).T
    bp = np.zeros((P, P), np.float16)          # proj bias rhs, row D = b_proj
    bp[D, :] = b_proj
    return {"wh": wh, "w4": w4, "wp": wp, "bp": bp}


# ---------------------------------------------------------------- bass builder
def build_nc(nt=NT, u=U, debug=False):
    import concourse.mybir as mybir
    import concourse.tile as tile
    from concourse import bacc
    from concourse.bass import ds
    from concourse.masks import make_identity
    from contextlib import ExitStack

    f32 = mybir.dt.float32
    f16 = mybir.dt.float16
    Tanh = mybir.ActivationFunctionType.Tanh
    Exp = mybir.ActivationFunctionType.Exp
    add = mybir.AluOpType.add
    mult = mybir.AluOpType.mult
    subtract = mybir.AluOpType.subtract

    assert nt % u == 0

    nc = bacc.Bacc("TRN2")
    h0_d = nc.declare_dram_parameter("h0", [B, H], f32, isOutput=False)
    c0_d = nc.declare_dram_parameter("c0", [B, H], f32, isOutput=False)
    y0_d = nc.declare_dram_parameter("y0", [B, D], f32, isOutput=False)
    eps_d = nc.declare_dram_parameter("eps", [B, nt, D], f32, isOutput=False)
    wh_d = nc.declare_dram_parameter("wh", [P, 4, 16, P], f16, isOutput=False)
    w4_d = nc.declare_dram_parameter("w4", [P, 16, P], f16, isOutput=False)
    wp_d = nc.declare_dram_parameter("wp", [P, 4, P], f16, isOutput=False)
    bp_d = nc.declare_dram_parameter("bp", [P, P], f16, isOutput=False)
    ys_d = nc.declare_dram_parameter("ys", [B, nt, D], f32, isOutput=True)
    mus_d = nc.declare_dram_parameter("mus", [B, nt, D], f32, isOutput=True)
    lvs_d = nc.declare_dram_parameter("lvs", [B, nt, D], f32, isOutput=True)
    if debug:
        dbg_gt = nc.declare_dram_parameter("dbg_gt", [P, 4, P], f16, isOutput=True)
        dbg_h = nc.declare_dram_parameter("dbg_h", [P, 4, 32], f16, isOutput=True)
        dbg_c = nc.declare_dram_parameter("dbg_c", [P, 4, 32], f32, isOutput=True)
        dbg_s4 = nc.declare_dram_parameter("dbg_s4", [P, 32], f16, isOutput=True)
        dbg_h0 = nc.declare_dram_parameter("dbg_h0", [P, 4, 32], f16, isOutput=True)

    with tile.TileContext(nc) as tc, ExitStack() as ctx:
        const = ctx.enter_context(tc.tile_pool(name="const", bufs=1))
        state = ctx.enter_context(tc.tile_pool(name="state", bufs=1))
        dyn = ctx.enter_context(tc.tile_pool(name="dyn", bufs=2))
        tmp = ctx.enter_context(tc.tile_pool(name="tmp", bufs=3))
        # PSUM bank budget (8): psg 4 gate banks + 1 warm-up, psp 2, pst 1.
        psg_pool = ctx.enter_context(tc.tile_pool(name="psg", bufs=1, space="PSUM"))
        psp_pool = ctx.enter_context(tc.tile_pool(name="psp", bufs=2, space="PSUM"))
        pst_pool = ctx.enter_context(tc.tile_pool(name="pst", bufs=1, space="PSUM"))

        # ---- constants
        wh_sb = const.tile([P, 4, 16, P], f16)
        nc.sync.dma_start(wh_sb, wh_d[:])
        w4_sb = const.tile([P, 16, P], f16)
        nc.sync.dma_start(w4_sb, w4_d[:])
        wp_sb = const.tile([P, 4, P], f16)
        nc.sync.dma_start(wp_sb, wp_d[:])
        bp_sb = const.tile([P, P], f16)
        nc.sync.dma_start(bp_sb, bp_d[:])
        ident = const.tile([32, 32], f32)
        make_identity(nc, ident)

        # ---- state
        hT = state.tile([P, 4, 32], f16)   # H = 2h, feature-major
        cT = state.tile([P, 4, 32], f32)   # C = 2c
        s4 = state.tile([P, 32], f16)      # [z(64); 1; 0(63)] moving chunk
        gt = state.tile([P, 4, P], f16)    # tanh(gates) [q][i f o g]
        tc_sb = state.tile([P, 4, 32], f16)  # tanh(c)
        psg = [psg_pool.tile([P, P], f32, tag=f"psg{q}", name=f"psg{q}") for q in range(4)]

        # ---- prologue: load & transpose initial state, z0 = y0 - mu(h0)
        h0_sb = tmp.tile([B, H], f32, tag="init_h")
        nc.sync.dma_start(h0_sb, h0_d[:])
        c0_sb = tmp.tile([B, H], f32, tag="init_c")
        nc.sync.dma_start(c0_sb, c0_d[:])
        y0_sb = tmp.tile([B, D], f32, tag="init_y")
        nc.sync.dma_start(y0_sb, y0_d[:])

        nc.vector.memset(s4, 0.0)
        nc.vector.memset(s4[D:D + 1, :], 1.0)

        # Warm-up transpose consuming only `ident`: the PE vector clock then
        # covers the gpsimd tick, so the h0/c0 transposes below carry a
        # single sync wait each (walrus's LDWEIGHTS struct holds only one).
        warm = psg_pool.tile([32, 32], f32, tag="warm")
        nc.tensor.transpose(warm, ident, ident)

        for q in range(4):
            pt = pst_pool.tile([P, 32], f32, tag="ztr")
            nc.tensor.transpose(pt, h0_sb[:, P * q:P * (q + 1)], ident)
            nc.vector.tensor_scalar_mul(hT[:, q, :], pt, 2.0)
            pt2 = pst_pool.tile([P, 32], f32, tag="ztr")
            nc.tensor.transpose(pt2, c0_sb[:, P * q:P * (q + 1)], ident)
            nc.vector.tensor_scalar_mul(cT[:, q, :], pt2, 2.0)

        ps_p0 = psp_pool.tile([B, P], f32, tag="psp")
        for kc in range(4):
            nc.tensor.matmul(ps_p0, hT[:, kc, :], wp_sb[:, kc, :],
                             start=(kc == 0), stop=False)
        nc.tensor.matmul(ps_p0, s4, bp_sb, start=False, stop=True)
        z0 = tmp.tile([B, D], f32, tag="zf")
        nc.vector.tensor_tensor(z0, y0_sb, ps_p0[:, 0:D], subtract)
        ztr0 = pst_pool.tile([D, 32], f32, tag="ztr")
        nc.tensor.transpose(ztr0, z0, ident)
        nc.vector.tensor_copy(s4[0:D, :], ztr0)
        if debug:
            nc.sync.dma_start(dbg_s4[:], s4)
            nc.sync.dma_start(dbg_h0[:], hT)

        # ---- main loop
        with tc.For_i(0, nt, u) as iv:
            eps_sb = dyn.tile([B, u, D], f32, tag="eps")
            nc.sync.dma_start(eps_sb, eps_d[:, ds(iv, u), :])
            y_st = dyn.tile([B, u, D], f32, tag="yst")
            mu_st = dyn.tile([B, u, D], f32, tag="must")
            lv_st = dyn.tile([B, u, D], f32, tag="lvst")

            for tt in range(u):
                # gates: 4 psum banks, chunk-major so ACT can chase the PE.
                # All MMs are issued before any hT update (the cell update
                # below overwrites hT, which every m-tile's matmuls read).
                for q in range(4):
                    for j in range(4):
                        m = 4 * q + j
                        o_ap = psg[q][:, 32 * j:32 * (j + 1)]
                        for kc in range(4):
                            nc.tensor.matmul(o_ap, wh_sb[:, kc, m, :],
                                             hT[:, kc, :],
                                             start=(kc == 0), stop=False)
                        nc.tensor.matmul(o_ap, w4_sb[:, m, :], s4,
                                         start=False, stop=True)
                for q in range(4):
                    # tanh over [i f o g] of this chunk
                    nc.scalar.activation(gt[:, q, :], psg[q], Tanh)
                    # cell update (fused): A=(ti+1)*tg; B=(tf+1)*C; C'=.5B+A
                    A = tmp.tile([P, 32], f32, tag="A")
                    nc.vector.scalar_tensor_tensor(
                        A, gt[:, q, 0:32], 1.0, gt[:, q, 96:128], add, mult)
                    Bt = tmp.tile([P, 32], f32, tag="B")
                    nc.vector.scalar_tensor_tensor(
                        Bt, gt[:, q, 32:64], 1.0, cT[:, q, :], add, mult)
                    nc.vector.scalar_tensor_tensor(
                        cT[:, q, :], Bt, 0.5, A, mult, add)
                    nc.scalar.activation(tc_sb[:, q, :], cT[:, q, :], Tanh,
                                         scale=0.5)
                    nc.vector.scalar_tensor_tensor(
                        hT[:, q, :], gt[:, q, 64:96], 1.0, tc_sb[:, q, :],
                        add, mult)

                # proj (batch-major out): [mu|lv] psum [32, 128]
                ps_p = psp_pool.tile([B, P], f32, tag="psp")
                for kc in range(4):
                    nc.tensor.matmul(ps_p, hT[:, kc, :], wp_sb[:, kc, :],
                                     start=(kc == 0), stop=False)
                nc.tensor.matmul(ps_p, s4, bp_sb, start=False, stop=True)

                # z path
                std = tmp.tile([B, D], f32, tag="std")
                nc.scalar.activation(std, ps_p[:, D:2 * D], Exp, scale=0.5)
                zf = tmp.tile([B, D], f32, tag="zf")
                nc.vector.tensor_tensor(zf, eps_sb[:, tt, :], std, mult)
                nc.vector.tensor_tensor(y_st[:, tt, :], zf, ps_p[:, 0:D], add)
                nc.scalar.copy(mu_st[:, tt, :], ps_p[:, 0:D])
                nc.scalar.copy(lv_st[:, tt, :], ps_p[:, D:2 * D])
                ztr = pst_pool.tile([D, 32], f32, tag="ztr")
                nc.tensor.transpose(ztr, zf, ident)
                nc.vector.tensor_copy(s4[0:D, :], ztr)

            if debug:
                nc.sync.dma_start(dbg_gt[:], gt)
                nc.sync.dma_start(dbg_h[:], hT)
                nc.sync.dma_start(dbg_c[:], cT)
            nc.sync.dma_start(ys_d[:, ds(iv, u), :], y_st)
            nc.sync.dma_start(mus_d[:, ds(iv, u), :], mu_st)
            nc.sync.dma_start(lvs_d[:, ds(iv, u), :], lv_st)

    if not nc.is_finalized():
        nc.finalize()
    return nc


# ---------------------------------------------------------------- entry point
_cache = {}


def run_kernel(h0, c0, yt, eps, W_ih, W_hh, b_ih, b_hh, W_proj, b_proj,
               trace=False, **spmd_kwargs):
    """Build (cached), shard, execute on 8 cores; returns (outputs, results)."""
    from concourse.bass_utils import run_bass_kernel_spmd

    h0 = np.asarray(h0, np.float32)
    c0 = np.asarray(c0, np.float32)
    yt = np.asarray(yt, np.float32)
    eps = np.asarray(eps, np.float32)

    if "nc" not in _cache:
        _cache["nc"] = build_nc(NT, U)
    nc = _cache["nc"]
    wd = fold_weights(W_ih, W_hh, b_ih, b_hh, W_proj, b_proj)

    in_maps = []
    for c in range(NCORES):
        sl = slice(B * c, B * (c + 1))
        in_maps.append({
            "h0": np.ascontiguousarray(h0[sl]),
            "c0": np.ascontiguousarray(c0[sl]),
            "y0": np.ascontiguousarray(yt[sl, 0, :]),
            "eps": np.ascontiguousarray(eps[sl]),
            **wd,
        })

    res = run_bass_kernel_spmd(nc, in_maps, core_ids=list(range(NCORES)),
                               trace=trace, **spmd_kwargs)
    ys = np.concatenate([res.results[c]["ys"] for c in range(NCORES)], axis=0)
    mus = np.concatenate([res.results[c]["mus"] for c in range(NCORES)], axis=0)
    lvs = np.concatenate([res.results[c]["lvs"] for c in range(NCORES)], axis=0)
    return (ys, mus, lvs), res


def kernel(input=None, h0=None, c0=None, yt=None, eps=None, W_ih=None,
           W_hh=None, b_ih=None, b_hh=None, W_proj=None, b_proj=None,
           **kwargs):
    out, _ = run_kernel(h0, c0, yt, eps, W_ih, W_hh, b_ih, b_hh,
                        W_proj, b_proj)
    return out


# revision 22
# speedup vs baseline: 1.0348x; 1.0348x over previous
"""AR-LSTM sampling kernel for 8 TRN2 NeuronCores.

nn_ARLSTMModel: 1024-step autoregressive LSTM rollout (H=512, D=64, bs=256),
data-parallel over batch (32 rows/core), weights replicated in SBUF.

Math (per step, per core, feature-major):
    gates = W~ @ h + W_ih @ z + b~          (W~ = W_hh + W_ih@W_pmu folds the
                                             mu-part of the y feedback;
                                             z = eps*std is the only feedback)
    i,f,o ~ sigmoid, g ~ tanh; sigmoid computed as 0.5*tanh(x/2)+0.5 so the
    whole step uses one ACT table set (tanh+exp).
    State is stored doubled (H=2h, C=2c) so the cell update becomes fused
    scalar_tensor_tensor ops:
        A  = (tanh_i + 1) * tanh_g
        B  = (tanh_f + 1) * C
        C' = 0.5*B + A
        H' = (tanh_o + 1) * tanh(0.5*C')
    with the 0.5 factors folded into the weights host-side.
    proj: [mu|lv] = 0.5*W_proj @ H + b_proj (bias via const `1` row in the
    moving state vector s4 = [z(64); 1; 0...]).
    y = z + mu,  z' = eps_t * exp(0.5*lv)
"""

import numpy as np

H, D, BS, NT = 512, 64, 256, 1024
NCORES = 8
B = BS // NCORES  # 32
P = 128
U = 16  # steps per For_i iteration


# ---------------------------------------------------------------- weights fold
def fold_weights(W_ih, W_hh, b_ih, b_hh, W_proj, b_proj):
    """Host-side fold + layout. Returns dict of fp16 arrays for DRAM params."""
    W_ih = np.asarray(W_ih, np.float32)
    W_hh = np.asarray(W_hh, np.float32)
    W_proj = np.asarray(W_proj, np.float32)
    b_proj = np.asarray(b_proj, np.float32)
    W_pmu = W_proj[:D]          # [64, 512]
    b_mu = b_proj[:D]
    Wt = W_hh + W_ih @ W_pmu    # [2048, 512]
    bt = np.asarray(b_ih, np.float32) + np.asarray(b_hh, np.float32) + W_ih @ b_mu

    # m-tile order is gate-type-major: m = 4j + q with banks [g, i, f, o],
    # so PSUM bank j holds one whole gate [128 rows x 4 chunks] and the cell
    # update runs as full-width [128,128] fused ops.
    goff = {0: 2 * H, 1: 0, 2: H, 3: 3 * H}    # g, i, f, o
    wh = np.zeros((P, 4, 16, P), np.float16)   # [K=128, kc, m, M=128]
    w4 = np.zeros((P, 16, P), np.float16)      # [K=128 (z,1,pad), m, M]
    for j in range(4):
        for q in range(4):
            m = 4 * j + q
            rows = slice(goff[j] + P * q, goff[j] + P * q + P)
            s = 1.0 if j == 0 else 0.5         # sigmoid-as-tanh prescale
            for kc in range(4):
                # extra 0.5: state stored as H=2h
                wh[:, kc, m, :] = (s * 0.5 * Wt[rows, P * kc:P * (kc + 1)]).T
            w4[:D, m, :] = (s * W_ih[rows, :]).T   # z enters unscaled
            w4[D, m, :] = s * bt[rows]             # bias row (hits the `1`)
    wp = np.zeros((P, 4, P), np.float16)       # proj moving [K=H chunk, kc, 2D]
    for kc in range(4):
        wp[:, kc, :] = (0.5 * W_proj[:, P * kc:P * (kc + 1)]).T
    bp = np.zeros((P, P), np.float16)          # proj bias rhs, row D = b_proj
    bp[D, :] = b_proj
    return {"wh": wh, "w4": w4, "wp": wp, "bp": bp}


# ---------------------------------------------------------------- bass builder
def build_nc(nt=NT, u=U, debug=False):
    import concourse.mybir as mybir
    import concourse.tile as tile
    from concourse import bacc
    from concourse.bass import ds
    from concourse.masks import make_identity
    from contextlib import ExitStack

    f32 = mybir.dt.float32
    f16 = mybir.dt.float16
    Tanh = mybir.ActivationFunctionType.Tanh
    Exp = mybir.ActivationFunctionType.Exp
    add = mybir.AluOpType.add
    mult = mybir.AluOpType.mult
    subtract = mybir.AluOpType.subtract

    assert nt % u == 0

    nc = bacc.Bacc("TRN2")
    h0_d = nc.declare_dram_parameter("h0", [B, H], f32, isOutput=False)
    c0_d = nc.declare_dram_parameter("c0", [B, H], f32, isOutput=False)
    y0_d = nc.declare_dram_parameter("y0", [B, D], f32, isOutput=False)
    eps_d = nc.declare_dram_parameter("eps", [B, nt, D], f32, isOutput=False)
    wh_d = nc.declare_dram_parameter("wh", [P, 4, 16, P], f16, isOutput=False)
    w4_d = nc.declare_dram_parameter("w4", [P, 16, P], f16, isOutput=False)
    wp_d = nc.declare_dram_parameter("wp", [P, 4, P], f16, isOutput=False)
    bp_d = nc.declare_dram_parameter("bp", [P, P], f16, isOutput=False)
    ys_d = nc.declare_dram_parameter("ys", [B, nt, D], f32, isOutput=True)
    mus_d = nc.declare_dram_parameter("mus", [B, nt, D], f32, isOutput=True)
    lvs_d = nc.declare_dram_parameter("lvs", [B, nt, D], f32, isOutput=True)
    if debug:
        dbg_gt = nc.declare_dram_parameter("dbg_gt", [P, 4, P], f16, isOutput=True)
        dbg_h = nc.declare_dram_parameter("dbg_h", [P, P], f16, isOutput=True)
        dbg_c = nc.declare_dram_parameter("dbg_c", [P, P], f32, isOutput=True)
        dbg_s4 = nc.declare_dram_parameter("dbg_s4", [P, 32], f16, isOutput=True)
        dbg_h0 = nc.declare_dram_parameter("dbg_h0", [P, P], f16, isOutput=True)

    with tile.TileContext(nc) as tc, ExitStack() as ctx:
        const = ctx.enter_context(tc.tile_pool(name="const", bufs=1))
        state = ctx.enter_context(tc.tile_pool(name="state", bufs=1))
        dyn = ctx.enter_context(tc.tile_pool(name="dyn", bufs=2))
        tmp = ctx.enter_context(tc.tile_pool(name="tmp", bufs=3))
        # PSUM bank budget (8): psg 4 gate banks + 1 warm-up, psp 2, pst 1.
        psg_pool = ctx.enter_context(tc.tile_pool(name="psg", bufs=1, space="PSUM"))
        psp_pool = ctx.enter_context(tc.tile_pool(name="psp", bufs=2, space="PSUM"))
        pst_pool = ctx.enter_context(tc.tile_pool(name="pst", bufs=1, space="PSUM"))

        # ---- constants
        wh_sb = const.tile([P, 4, 16, P], f16)
        nc.sync.dma_start(wh_sb, wh_d[:])
        w4_sb = const.tile([P, 16, P], f16)
        nc.sync.dma_start(w4_sb, w4_d[:])
        wp_sb = const.tile([P, 4, P], f16)
        nc.sync.dma_start(wp_sb, wp_d[:])
        bp_sb = const.tile([P, P], f16)
        nc.sync.dma_start(bp_sb, bp_d[:])
        ident = const.tile([32, 32], f32)
        make_identity(nc, ident)

        # ---- state (feature-major; free dim 128 = 4 H-chunks of 32 batch)
        hT = state.tile([P, P], f16)       # H = 2h
        cT = state.tile([P, P], f32)       # C = 2c
        s4 = state.tile([P, 32], f16)      # [z(64); 1; 0(63)] moving chunk
        gt = state.tile([P, 4, P], f16)    # tanh(gates), bank-major [g i f o]
        tc_sb = state.tile([P, P], f16)    # tanh(c)
        psg = [psg_pool.tile([P, P], f32, tag=f"psg{j}", name=f"psg{j}") for j in range(4)]

        # ---- prologue: load & transpose initial state, z0 = y0 - mu(h0)
        h0_sb = tmp.tile([B, H], f32, tag="init_h")
        nc.sync.dma_start(h0_sb, h0_d[:])
        c0_sb = tmp.tile([B, H], f32, tag="init_c")
        nc.sync.dma_start(c0_sb, c0_d[:])
        y0_sb = tmp.tile([B, D], f32, tag="init_y")
        nc.sync.dma_start(y0_sb, y0_d[:])

        nc.vector.memset(s4, 0.0)
        nc.vector.memset(s4[D:D + 1, :], 1.0)

        # Warm-up transpose consuming only `ident`: the PE vector clock then
        # covers the gpsimd tick, so the h0/c0 transposes below carry a
        # single sync wait each (walrus's LDWEIGHTS struct holds only one).
        warm = psg_pool.tile([32, 32], f32, tag="warm")
        nc.tensor.transpose(warm, ident, ident)

        for q in range(4):
            pt = pst_pool.tile([P, 32], f32, tag="ztr")
            nc.tensor.transpose(pt, h0_sb[:, P * q:P * (q + 1)], ident)
            nc.vector.tensor_scalar_mul(hT[:, 32 * q:32 * (q + 1)], pt, 2.0)
            pt2 = pst_pool.tile([P, 32], f32, tag="ztr")
            nc.tensor.transpose(pt2, c0_sb[:, P * q:P * (q + 1)], ident)
            nc.vector.tensor_scalar_mul(cT[:, 32 * q:32 * (q + 1)], pt2, 2.0)

        ps_p0 = psp_pool.tile([B, P], f32, tag="psp")
        for kc in range(4):
            nc.tensor.matmul(ps_p0, hT[:, 32 * kc:32 * (kc + 1)], wp_sb[:, kc, :],
                             start=(kc == 0), stop=False)
        nc.tensor.matmul(ps_p0, s4, bp_sb, start=False, stop=True)
        z0 = tmp.tile([B, D], f32, tag="zf")
        nc.vector.tensor_tensor(z0, y0_sb, ps_p0[:, 0:D], subtract)
        ztr0 = pst_pool.tile([D, 32], f32, tag="ztr")
        nc.tensor.transpose(ztr0, z0, ident)
        nc.vector.tensor_copy(s4[0:D, :], ztr0)
        if debug:
            nc.sync.dma_start(dbg_s4[:], s4)
            nc.sync.dma_start(dbg_h0[:], hT)

        # ---- main loop
        with tc.For_i(0, nt, u) as iv:
            eps_sb = dyn.tile([B, u, D], f32, tag="eps")
            nc.sync.dma_start(eps_sb, eps_d[:, ds(iv, u), :])
            y_st = dyn.tile([B, u, D], f32, tag="yst")
            mu_st = dyn.tile([B, u, D], f32, tag="must")
            lv_st = dyn.tile([B, u, D], f32, tag="lvst")

            for tt in range(u):
                # gates: 4 psum banks, gate-type-major [g, i, f, o].
                # Bank order lets the cell update chase the PE sweep:
                # tanh_g/A after bank 0, B/C'/tanh_c during bank 3 (o),
                # H right after tanh_o. All MMs are issued before the hT
                # update (every m-tile's matmuls read hT).
                for j in range(4):
                    # One accumulation group per bank: start=True marks the
                    # whole 2KB zero region; per-element has_written bits then
                    # give overwrite-on-first-touch / accumulate-after for
                    # every column slice.
                    for q in range(4):
                        m = 4 * j + q
                        o_ap = psg[j][:, 32 * q:32 * (q + 1)]
                        for kc in range(4):
                            nc.tensor.matmul(o_ap, wh_sb[:, kc, m, :],
                                             hT[:, 32 * kc:32 * (kc + 1)],
                                             start=(q == 0 and kc == 0),
                                             stop=False)
                    for q in range(4):
                        m = 4 * j + q
                        nc.tensor.matmul(psg[j][:, 32 * q:32 * (q + 1)],
                                         w4_sb[:, m, :], s4,
                                         start=False, stop=(q == 3))
                    # tanh of this gate (full [128, 128])
                    nc.scalar.activation(gt[:, j, :], psg[j], Tanh)
                    if j == 1:    # A = (tanh_i + 1) * tanh_g
                        A = tmp.tile([P, P], f32, tag="A")
                        nc.vector.scalar_tensor_tensor(
                            A, gt[:, 1, :], 1.0, gt[:, 0, :], add, mult)
                    elif j == 2:  # B = (tanh_f + 1) * C ; C' = 0.5*B + A
                        Bt = tmp.tile([P, P], f32, tag="B")
                        nc.vector.scalar_tensor_tensor(
                            Bt, gt[:, 2, :], 1.0, cT, add, mult)
                        nc.vector.scalar_tensor_tensor(
                            cT, Bt, 0.5, A, mult, add)
                        nc.scalar.activation(tc_sb, cT, Tanh, scale=0.5)
                    elif j == 3:  # H = (tanh_o + 1) * tanh_c
                        nc.vector.scalar_tensor_tensor(
                            hT, gt[:, 3, :], 1.0, tc_sb, add, mult)

                # proj (batch-major out): [mu|lv] psum [32, 128]
                ps_p = psp_pool.tile([B, P], f32, tag="psp")
                for kc in range(4):
                    nc.tensor.matmul(ps_p, hT[:, 32 * kc:32 * (kc + 1)],
                                     wp_sb[:, kc, :],
                                     start=(kc == 0), stop=False)
                nc.tensor.matmul(ps_p, s4, bp_sb, start=False, stop=True)

                # z path
                std = tmp.tile([B, D], f32, tag="std")
                nc.scalar.activation(std, ps_p[:, D:2 * D], Exp, scale=0.5)
                zf = tmp.tile([B, D], f32, tag="zf")
                nc.vector.tensor_tensor(zf, eps_sb[:, tt, :], std, mult)
                nc.vector.tensor_tensor(y_st[:, tt, :], zf, ps_p[:, 0:D], add)
                nc.scalar.copy(mu_st[:, tt, :], ps_p[:, 0:D])
                nc.vector.tensor_copy(lv_st[:, tt, :], ps_p[:, D:2 * D])
                ztr = pst_pool.tile([D, 32], f32, tag="ztr")
                nc.tensor.transpose(ztr, zf, ident)
                nc.vector.tensor_copy(s4[0:D, :], ztr)

            if debug:
                nc.sync.dma_start(dbg_gt[:], gt)
                nc.sync.dma_start(dbg_h[:], hT)
                nc.sync.dma_start(dbg_c[:], cT)
            nc.sync.dma_start(ys_d[:, ds(iv, u), :], y_st)
            nc.sync.dma_start(mus_d[:, ds(iv, u), :], mu_st)
            nc.sync.dma_start(lvs_d[:, ds(iv, u), :], lv_st)

    if not nc.is_finalized():
        nc.finalize()
    return nc


# ---------------------------------------------------------------- entry point
_cache = {}


def run_kernel(h0, c0, yt, eps, W_ih, W_hh, b_ih, b_hh, W_proj, b_proj,
               trace=False, **spmd_kwargs):
    """Build (cached), shard, execute on 8 cores; returns (outputs, results)."""
    from concourse.bass_utils import run_bass_kernel_spmd

    h0 = np.asarray(h0, np.float32)
    c0 = np.asarray(c0, np.float32)
    yt = np.asarray(yt, np.float32)
    eps = np.asarray(eps, np.float32)

    if "nc" not in _cache:
        _cache["nc"] = build_nc(NT, U)
    nc = _cache["nc"]
    wd = fold_weights(W_ih, W_hh, b_ih, b_hh, W_proj, b_proj)

    in_maps = []
    for c in range(NCORES):
        sl = slice(B * c, B * (c + 1))
        in_maps.append({
            "h0": np.ascontiguousarray(h0[sl]),
            "c0": np.ascontiguousarray(c0[sl]),
            "y0": np.ascontiguousarray(yt[sl, 0, :]),
            "eps": np.ascontiguousarray(eps[sl]),
            **wd,
        })

    res = run_bass_kernel_spmd(nc, in_maps, core_ids=list(range(NCORES)),
                               trace=trace, **spmd_kwargs)
    ys = np.concatenate([res.results[c]["ys"] for c in range(NCORES)], axis=0)
    mus = np.concatenate([res.results[c]["mus"] for c in range(NCORES)], axis=0)
    lvs = np.concatenate([res.results[c]["lvs"] for c in range(NCORES)], axis=0)
    return (ys, mus, lvs), res


def kernel(input=None, h0=None, c0=None, yt=None, eps=None, W_ih=None,
           W_hh=None, b_ih=None, b_hh=None, W_proj=None, b_proj=None,
           **kwargs):
    out, _ = run_kernel(h0, c0, yt, eps, W_ih, W_hh, b_ih, b_hh,
                        W_proj, b_proj)
    return out
